# revision 2
# baseline (speedup 1.0000x reference)
"""MoE-LoRA linear layer (T=16384, D=1024, E=64, R=8) on 8 Trainium2 cores.

Strategy: data-parallel over tokens (2048 tokens/core). Inside each core:
  out_T = W^T-free base GEMM + per-token rank-8 LoRA correction, all
  computed transposed (d on partitions, tokens on the free dim) so every
  matmul consumes operands in their natural layout with no on-device
  transposes.

Routing is resolved on the host: each core's tokens are sorted by expert
label and cut into fixed 256-token blocks. For each block the host packs
the (<=16) experts present into per-block A / B / mask tensors, so the
device program is identical for all 8 cores (one SPMD NEFF) and all
data-dependence lives in the input data:

  xa_blk   = A_blk^T @ xT_blk           (PE, K=1024, 128 expert-slot rows)
  xa_m     = xa_blk * mask_blk          (DVE; zeroes wrong-expert slots,
                                         casts to bf16)
  psum     = sum_k W_k^T @ xT_blk_k     (PE, base GEMM accumulation)
  psum    += B_blk^T @ xa_m             (PE, LoRA folded into same PSUM)
  out      = psum + bias                (DVE, PSUM -> SBUF)

Compute in bf16 (f32 accumulation in PSUM): fp32 matmul on TRN2 runs at
1/4 rate and would be hopelessly PE-bound; bf16 also halves DMA traffic.
"""

import numpy as np
import ml_dtypes

import concourse.bacc as bacc
import concourse.mybir as mybir
from concourse import tile
from concourse.bass_utils import run_bass_kernel_spmd

T, D, E, R = 16384, 1024, 64, 8
N_CORES = 8
TPC = T // N_CORES          # tokens per core
KD = D // 128               # 8 contraction chunks
SCALING = 1.0 / R
SLOTS = 128 // R            # experts per block the packed layout can hold

BF16 = ml_dtypes.bfloat16

_compiled = {}              # n_blocks -> Bacc program (reused across calls)


def _build_nc(n_blocks: int):
    blk = TPC // n_blocks
    assert blk <= 512
    bf = mybir.dt.bfloat16
    f32 = mybir.dt.float32

    nc = bacc.Bacc(
        "TRN2", target_bir_lowering=False, debug=False, num_devices=N_CORES
    )
    xT_d = nc.dram_tensor("xT", [n_blocks, KD, 128, blk], bf, kind="ExternalInput")
    w_d = nc.dram_tensor("W", [KD, 128, D], bf, kind="ExternalInput")
    a_d = nc.dram_tensor("Ab", [n_blocks, 128, D], bf, kind="ExternalInput")
    b_d = nc.dram_tensor("Bb", [n_blocks, 128, D], bf, kind="ExternalInput")
    m_d = nc.dram_tensor("Mb", [n_blocks, 128, blk], bf, kind="ExternalInput")
    bias_d = nc.dram_tensor("bias", [128, KD], f32, kind="ExternalInput")
    out_d = nc.dram_tensor("outT", [KD, 128, TPC], f32, kind="ExternalOutput")

    with tile.TileContext(nc) as tc:
        with (
            tc.tile_pool(name="consts", bufs=1) as cpool,
            tc.tile_pool(name="xa_ps", bufs=2, space="PSUM") as xa_ps,
            tc.tile_pool(name="out_ps", bufs=4, space="PSUM") as out_ps,
            tc.tile_pool(name="xa_sb", bufs=3) as xa_pool,
            tc.tile_pool(name="out_sb", bufs=6) as osb_pool,
        ):
            bias_t = cpool.tile([128, KD], f32, tag="bias")
            nc.sync.dma_start(bias_t[:], bias_d[:, :])

            w_t = []
            for k in range(KD):
                t = cpool.tile([128, D], bf, tag=f"w{k}")
                nc.sync.dma_start(t[:], w_d[k, :, :])
                w_t.append(t)

            a_t, b_t, m_t = [], [], []
            for b in range(n_blocks):
                ta = cpool.tile([128, D], bf, tag=f"a{b}")
                nc.sync.dma_start(ta[:], a_d[b, :, :])
                tb = cpool.tile([128, D], bf, tag=f"b{b}")
                nc.sync.dma_start(tb[:], b_d[b, :, :])
                tm = cpool.tile([128, blk], bf, tag=f"m{b}")
                nc.sync.dma_start(tm[:], m_d[b, :, :])
                a_t.append(ta)
                b_t.append(tb)
                m_t.append(tm)

            x_t = []
            for b in range(n_blocks):
                per_k = []
                for k in range(KD):
                    t = cpool.tile([128, blk], bf, tag=f"x{b}_{k}")
                    nc.sync.dma_start(t[:], xT_d[b, k, :, :])
                    per_k.append(t)
                x_t.append(per_k)

            for b in range(n_blocks):
                # xa[slot, t] for all expert slots of this block
                xa_p = xa_ps.tile([128, blk], f32, tag="xa")
                for k in range(KD):
                    nc.tensor.matmul(
                        xa_p[:],
                        lhsT=a_t[b][:, k * 128 : (k + 1) * 128],
                        rhs=x_t[b][k][:],
                        start=(k == 0),
                        stop=(k == KD - 1),
                    )
                xa_m = xa_pool.tile([128, blk], bf, tag="xam")
                nc.vector.tensor_mul(xa_m[:], xa_p[:], m_t[b][:])

                for j in range(KD):
                    o_p = out_ps.tile([128, blk], f32, tag="o")
                    for k in range(KD):
                        nc.tensor.matmul(
                            o_p[:],
                            lhsT=w_t[k][:, j * 128 : (j + 1) * 128],
                            rhs=x_t[b][k][:],
                            start=(k == 0),
                            stop=False,
                            skip_group_check=True,
                        )
                    nc.tensor.matmul(
                        o_p[:],
                        lhsT=b_t[b][:, j * 128 : (j + 1) * 128],
                        rhs=xa_m[:],
                        start=False,
                        stop=True,
                        skip_group_check=True,
                    )
                    o_sb = osb_pool.tile([128, blk], f32, tag="osb")
                    nc.vector.tensor_scalar_add(o_sb[:], o_p[:], bias_t[:, j : j + 1])
                    nc.sync.dma_start(out_d[j, :, b * blk : (b + 1) * blk], o_sb[:])

    nc.compile()
    return nc


def _pick_n_blocks(labels: np.ndarray) -> int:
    for n_blocks in (8, 16, 32, 64, 128, 256):
        blk = TPC // n_blocks
        ok = True
        for c in range(N_CORES):
            sl = np.sort(labels[c * TPC : (c + 1) * TPC])
            for b in range(n_blocks):
                if len(np.unique(sl[b * blk : (b + 1) * blk])) > SLOTS:
                    ok = False
                    break
            if not ok:
                break
        if ok:
            return n_blocks
    raise ValueError("could not find a block size with <=16 experts per block")


def kernel(x, labels, W, A, B, bias):
    x = np.asarray(x, dtype=np.float32)
    labels_i = np.asarray(labels).astype(np.int64)
    W = np.asarray(W, dtype=np.float32)
    A = np.asarray(A, dtype=np.float32)
    B = np.asarray(B, dtype=np.float32)
    bias = np.asarray(bias, dtype=np.float32)

    n_blocks = _pick_n_blocks(labels_i)
    blk = TPC // n_blocks

    if n_blocks not in _compiled:
        _compiled[n_blocks] = _build_nc(n_blocks)
    nc = _compiled[n_blocks]

    w_in = np.ascontiguousarray(W.reshape(KD, 128, D).astype(BF16))
    bias_in = np.ascontiguousarray(bias.reshape(KD, 128).T)  # [128, KD] f32
    B_scaled = (B * SCALING).astype(np.float32)

    in_maps = []
    perms = []
    for c in range(N_CORES):
        lc = labels_i[c * TPC : (c + 1) * TPC]
        perm = np.argsort(lc, kind="stable")
        perms.append(perm)
        ls = lc[perm]                       # sorted labels
        xs = x[c * TPC : (c + 1) * TPC][perm]  # [TPC, D] sorted tokens

        # xT in [n_blocks, KD, 128, blk]: xT[b,k,p,t] = x_sorted[b*blk+t, 128k+p]
        xT = np.ascontiguousarray(
            xs.astype(BF16).T.reshape(KD, 128, n_blocks, blk).transpose(2, 0, 1, 3)
        )

        a_in = np.zeros((n_blocks, 128, D), dtype=BF16)
        b_in = np.zeros((n_blocks, 128, D), dtype=BF16)
        m_in = np.zeros((n_blocks, 128, blk), dtype=BF16)
        for b in range(n_blocks):
            seg = ls[b * blk : (b + 1) * blk]
            experts = np.unique(seg)
            assert len(experts) <= SLOTS
            for i, e in enumerate(experts):
                # A slot: lhsT layout a_in[b, p, 128k + 8i + r] = A[e, 128k+p, r]
                ae = A[e].reshape(KD, 128, R)  # [k, p, r]
                a_block = a_in[b].reshape(128, KD, SLOTS, R)
                a_block[:, :, i, :] = ae.transpose(1, 0, 2)
                b_in[b, i * R : (i + 1) * R, :] = B_scaled[e]
                m_in[b, i * R : (i + 1) * R, :] = (seg == e)[None, :]

        in_maps.append(
            {
                "xT": xT,
                "W": w_in,
                "Ab": a_in,
                "Bb": b_in,
                "Mb": m_in,
                "bias": bias_in,
            }
        )

    global _last_in_maps
    _last_in_maps = in_maps
    res = run_bass_kernel_spmd(nc, in_maps, core_ids=list(range(N_CORES)))

    out = np.empty((T, D), dtype=np.float32)
    for c in range(N_CORES):
        o_t = res.results[c]["outT"].reshape(D, TPC)  # [d, t] sorted tokens
        out[c * TPC + perms[c]] = o_t.T
    return out


# revision 3
# speedup vs baseline: 1.2197x; 1.2197x over previous
"""MoE-LoRA linear layer (T=16384, D=1024, E=64, R=8) on 8 Trainium2 cores.

Strategy: data-parallel over tokens (2048 tokens/core). Inside each core
everything is computed transposed (d on partitions, tokens on the free dim)
so every matmul consumes operands in their natural layout with no on-device
transposes:

  out_T[:, g] = sum_k W_k^T @ xT_k[:, g]      base GEMM, N=512 token groups
  out_T[:, b] += B_blk^T @ (mask_b * (A_blk^T @ xT[:, b]))   rank-8 LoRA

Routing is resolved on the host: each core's tokens are sorted by expert
label and cut into 256-token blocks; per block the (<=16) experts present
are packed into per-block A / B / mask tensors. The device program is thus
identical for all 8 cores (one SPMD NEFF) and all data-dependence lives in
input data. The LoRA matmul accumulates directly into the base GEMM's PSUM
tile (column sub-range), so composition costs no extra DVE work.

All inputs are packed on the host into SBUF-resident layouts so each tensor
needs one large DMA (sequencer descriptor generation, ~5ns/descriptor, was
the v1 bottleneck at 161 small DMAs). Compute in bf16 (f32 PSUM): fp32
matmul on TRN2 runs at 1/4 rate and would be hopelessly PE-bound; bf16 also
halves DMA traffic.
"""

import numpy as np
import ml_dtypes

import concourse.bacc as bacc
import concourse.mybir as mybir
from concourse import tile
from concourse.bass_utils import run_bass_kernel_spmd

T, D, E, R = 16384, 1024, 64, 8
N_CORES = 8
TPC = T // N_CORES          # tokens per core
KD = D // 128               # 8 contraction chunks
GRP = 512                   # base-GEMM token group (one PSUM bank)
NG = TPC // GRP             # 4 groups
SCALING = 1.0 / R
SLOTS = 128 // R            # experts per lora block the packed layout holds

BF16 = ml_dtypes.bfloat16

_compiled = {}              # n_blocks -> Bacc program (reused across calls)
_last_in_maps = None


def _build_nc(n_blocks: int):
    blk = TPC // n_blocks   # lora block (256 default)
    sub = GRP // blk        # lora blocks per token group
    bf = mybir.dt.bfloat16
    f32 = mybir.dt.float32

    nc = bacc.Bacc(
        "TRN2", target_bir_lowering=False, debug=False, num_devices=N_CORES
    )
    # host-packed SBUF layouts, one DMA each
    xT_d = nc.dram_tensor("xT", [NG, 128, KD, GRP], bf, kind="ExternalInput")
    w_d = nc.dram_tensor("W", [128, KD, D], bf, kind="ExternalInput")
    a_d = nc.dram_tensor("Ab", [128, n_blocks, KD, 128], bf, kind="ExternalInput")
    b_d = nc.dram_tensor("Bb", [128, n_blocks, D], bf, kind="ExternalInput")
    m_d = nc.dram_tensor("Mb", [128, n_blocks, blk], bf, kind="ExternalInput")
    bias_d = nc.dram_tensor("bias", [128, KD], f32, kind="ExternalInput")
    out_d = nc.dram_tensor("outT", [KD, 128, TPC], f32, kind="ExternalOutput")

    with tile.TileContext(nc) as tc:
        with (
            tc.tile_pool(name="consts", bufs=1) as cpool,
            tc.tile_pool(name="xa_ps", bufs=3, space="PSUM") as xa_ps,
            tc.tile_pool(name="out_ps", bufs=4, space="PSUM") as out_ps,
            tc.tile_pool(name="stage", bufs=2) as stage_pool,
        ):
            bias_t = cpool.tile([128, KD], f32, tag="bias")
            nc.sync.dma_start(bias_t[:], bias_d[:, :])

            a_t = cpool.tile([128, n_blocks * KD * 128], bf, tag="a")
            nc.sync.dma_start(a_t[:], a_d[:, :, :, :])

            x_t = []
            for g in range(NG):
                t = cpool.tile([128, KD * GRP], bf, tag=f"x{g}")
                x_t.append(t)
            w_t = cpool.tile([128, KD * D], bf, tag="w")
            m_t = cpool.tile([128, n_blocks * blk], bf, tag="m")
            b_t = cpool.tile([128, n_blocks * D], bf, tag="b")

            # arrival order tuned for the dependency chain
            nc.sync.dma_start(x_t[0][:], xT_d[0, :, :, :])
            nc.sync.dma_start(w_t[:], w_d[:, :, :])
            nc.sync.dma_start(m_t[:], m_d[:, :, :])
            nc.sync.dma_start(x_t[1][:], xT_d[1, :, :, :])
            nc.sync.dma_start(b_t[:], b_d[:, :, :])
            nc.sync.dma_start(x_t[2][:], xT_d[2, :, :, :])
            nc.sync.dma_start(x_t[3][:], xT_d[3, :, :, :])

            # xa[slot, t] per lora block, masked to own-expert slots
            xa_m = []
            for b in range(n_blocks):
                g, h = divmod(b, sub)
                xa_p = xa_ps.tile([128, blk], f32, tag="xa")
                for k in range(KD):
                    nc.tensor.matmul(
                        xa_p[:],
                        lhsT=a_t[:, (b * KD + k) * 128 : (b * KD + k + 1) * 128],
                        rhs=x_t[g][:, k * GRP + h * blk : k * GRP + (h + 1) * blk],
                        start=(k == 0),
                        stop=(k == KD - 1),
                    )
                xm = cpool.tile([128, blk], bf, tag=f"xam{b}")
                nc.vector.tensor_mul(
                    xm[:], xa_p[:], m_t[:, b * blk : (b + 1) * blk]
                )
                xa_m.append(xm)

            for j in range(KD):
                stage = stage_pool.tile([128, TPC], f32, tag="stage")
                for g in range(NG):
                    o_p = out_ps.tile([128, GRP], f32, tag="o")
                    for k in range(KD):
                        nc.tensor.matmul(
                            o_p[:],
                            lhsT=w_t[:, k * D + j * 128 : k * D + (j + 1) * 128],
                            rhs=x_t[g][:, k * GRP : (k + 1) * GRP],
                            start=(k == 0),
                            stop=False,
                            skip_group_check=True,
                        )
                    for h in range(sub):
                        b = g * sub + h
                        nc.tensor.matmul(
                            o_p[:, h * blk : (h + 1) * blk],
                            lhsT=b_t[:, b * D + j * 128 : b * D + (j + 1) * 128],
                            rhs=xa_m[b][:],
                            start=False,
                            stop=(h == sub - 1),
                            skip_group_check=True,
                        )
                    nc.vector.tensor_scalar_add(
                        stage[:, g * GRP : (g + 1) * GRP], o_p[:], bias_t[:, j : j + 1]
                    )
                nc.sync.dma_start(out_d[j, :, :], stage[:])

    nc.compile()
    return nc


def _pick_n_blocks(labels: np.ndarray) -> int:
    for n_blocks in (8, 16, 32, 64, 128, 256):
        blk = TPC // n_blocks
        ok = True
        for c in range(N_CORES):
            sl = np.sort(labels[c * TPC : (c + 1) * TPC])
            for b in range(n_blocks):
                if len(np.unique(sl[b * blk : (b + 1) * blk])) > SLOTS:
                    ok = False
                    break
            if not ok:
                break
        if ok:
            return n_blocks
    raise ValueError("could not find a block size with <=16 experts per block")


def kernel(x, labels, W, A, B, bias):
    global _last_in_maps
    x = np.asarray(x, dtype=np.float32)
    labels_i = np.asarray(labels).astype(np.int64)
    W = np.asarray(W, dtype=np.float32)
    A = np.asarray(A, dtype=np.float32)
    B = np.asarray(B, dtype=np.float32)
    bias = np.asarray(bias, dtype=np.float32)

    n_blocks = _pick_n_blocks(labels_i)
    blk = TPC // n_blocks

    if n_blocks not in _compiled:
        _compiled[n_blocks] = _build_nc(n_blocks)
    nc = _compiled[n_blocks]

    # W[p, k, j] = W[128k+p, j]
    w_in = np.ascontiguousarray(W.reshape(KD, 128, D).transpose(1, 0, 2).astype(BF16))
    bias_in = np.ascontiguousarray(bias.reshape(KD, 128).T)  # [128, KD] f32
    B_scaled = (B * SCALING).astype(np.float32)

    in_maps = []
    perms = []
    for c in range(N_CORES):
        lc = labels_i[c * TPC : (c + 1) * TPC]
        perm = np.argsort(lc, kind="stable")
        perms.append(perm)
        ls = lc[perm]                          # sorted labels
        xs = x[c * TPC : (c + 1) * TPC][perm]  # [TPC, D] sorted tokens

        # xT[g, p, k, t] = xs[g*GRP + t, 128k + p]
        xT = np.ascontiguousarray(
            xs.astype(BF16).T.reshape(KD, 128, NG, GRP).transpose(2, 1, 0, 3)
        )

        a_in = np.zeros((128, n_blocks, KD, 128), dtype=BF16)
        b_in = np.zeros((128, n_blocks, D), dtype=BF16)
        m_in = np.zeros((128, n_blocks, blk), dtype=BF16)
        for b in range(n_blocks):
            seg = ls[b * blk : (b + 1) * blk]
            experts = np.unique(seg)
            assert len(experts) <= SLOTS
            for i, e in enumerate(experts):
                # lhsT slot: a_in[p, b, k, 8i+r] = A[e, 128k+p, r]
                a_in[:, b, :, i * R : (i + 1) * R] = A[e].reshape(KD, 128, R).transpose(
                    1, 0, 2
                )
                b_in[i * R : (i + 1) * R, b, :] = B_scaled[e]
                m_in[i * R : (i + 1) * R, b, :] = (seg == e)[None, :]

        in_maps.append(
            {
                "xT": xT,
                "W": w_in,
                "Ab": a_in,
                "Bb": b_in,
                "Mb": m_in,
                "bias": bias_in,
            }
        )

    _last_in_maps = in_maps
    res = run_bass_kernel_spmd(nc, in_maps, core_ids=list(range(N_CORES)))

    out = np.empty((T, D), dtype=np.float32)
    for c in range(N_CORES):
        o_t = res.results[c]["outT"].reshape(D, TPC)  # [d, t] sorted tokens
        out[c * TPC + perms[c]] = o_t.T
    return out


# revision 5
# speedup vs baseline: 1.3646x; 1.1188x over previous
"""MoE-LoRA linear layer (T=16384, D=1024, E=64, R=8) on 8 Trainium2 cores.

Strategy: data-parallel over tokens (2048 tokens/core). Inside each core
everything is computed transposed (d on partitions, tokens on the free dim)
so every matmul consumes operands in their natural layout with no on-device
transposes:

  out_T[:, g] = sum_k W_k^T @ xT_k[:, g]      base GEMM, N=512 token groups
  out_T[:, b] += B_blk^T @ (mask_b * (A_blk^T @ xT[:, b]))   rank-8 LoRA

Routing is resolved on the host: each core's tokens are sorted by expert
label and cut into 256-token blocks; per block the (<=16) experts present
are packed into per-block A / B / mask tensors. The device program is thus
identical for all 8 cores (one SPMD NEFF) and all data-dependence lives in
input data. The LoRA matmul accumulates directly into the base GEMM's PSUM
tile (column sub-range), so composition costs no extra DVE work.

All inputs are packed on the host into SBUF-resident layouts so each tensor
needs one large DMA (sequencer descriptor generation, ~5ns/descriptor, was
the v1 bottleneck at 161 small DMAs). Compute in bf16 (f32 PSUM): fp32
matmul on TRN2 runs at 1/4 rate and would be hopelessly PE-bound; bf16 also
halves DMA traffic.
"""

import numpy as np
import ml_dtypes

import concourse.bacc as bacc
import concourse.mybir as mybir
from concourse import tile
from concourse.bass_utils import run_bass_kernel_spmd

T, D, E, R = 16384, 1024, 64, 8
N_CORES = 8
TPC = T // N_CORES          # tokens per core
KD = D // 128               # 8 contraction chunks
GRP = 512                   # base-GEMM token group (one PSUM bank)
NG = TPC // GRP             # 4 groups
SCALING = 1.0 / R
SLOTS = 128 // R            # experts per lora block the packed layout holds

BF16 = ml_dtypes.bfloat16

_compiled = {}              # n_blocks -> Bacc program (reused across calls)
_last_in_maps = None


def _build_nc(n_blocks: int):
    blk = TPC // n_blocks   # lora block (256 default)
    sub = GRP // blk        # lora blocks per token group
    bf = mybir.dt.bfloat16
    f32 = mybir.dt.float32

    nc = bacc.Bacc(
        "TRN2", target_bir_lowering=False, debug=False, num_devices=N_CORES
    )
    # host-packed SBUF layouts, one DMA each
    xT_d = nc.dram_tensor("xT", [NG, 128, KD, GRP], bf, kind="ExternalInput")
    w_d = nc.dram_tensor("W", [128, KD, D], bf, kind="ExternalInput")
    a_d = nc.dram_tensor("Ab", [128, n_blocks, KD, 128], bf, kind="ExternalInput")
    b_d = nc.dram_tensor("Bb", [128, n_blocks, D], bf, kind="ExternalInput")
    m_d = nc.dram_tensor("Mb", [128, n_blocks, blk], bf, kind="ExternalInput")
    bias_d = nc.dram_tensor("bias", [128, KD], f32, kind="ExternalInput")
    out_d = nc.dram_tensor("outT", [KD, 128, TPC], f32, kind="ExternalOutput")

    with tile.TileContext(nc) as tc:
        with (
            tc.tile_pool(name="consts", bufs=1) as cpool,
            tc.tile_pool(name="xa_ps", bufs=2, space="PSUM") as xa_ps,
            tc.tile_pool(name="out_ps", bufs=5, space="PSUM") as out_ps,
            tc.tile_pool(name="stage", bufs=4) as stage_pool,
        ):
            bias_t = cpool.tile([128, KD], f32, tag="bias", name="bias_t")
            a_t = cpool.tile([128, n_blocks * KD * 128], bf, tag="a", name="a_t")
            x_t = [cpool.tile([128, KD * GRP], bf, tag=f"x{g}", name=f"x_t{g}") for g in range(NG)]
            w_t = cpool.tile([128, KD * D], bf, tag="w", name="w_t")
            m_t = cpool.tile([128, n_blocks * blk], bf, tag="m", name="m_t")
            b_t = cpool.tile([128, n_blocks * D], bf, tag="b", name="b_t")

            # issue order == arrival order (one sequencer queue): what the
            # PE needs first goes first
            nc.sync.dma_start(bias_t[:], bias_d[:, :])
            nc.sync.dma_start(a_t[:], a_d[:, :, :, :])
            nc.sync.dma_start(x_t[0][:], xT_d[0, :, :, :])
            nc.sync.dma_start(w_t[:], w_d[:, :, :])
            nc.sync.dma_start(m_t[:], m_d[:, :, :])
            nc.sync.dma_start(b_t[:], b_d[:, :, :])
            nc.sync.dma_start(x_t[1][:], xT_d[1, :, :, :])
            nc.sync.dma_start(x_t[2][:], xT_d[2, :, :, :])
            nc.sync.dma_start(x_t[3][:], xT_d[3, :, :, :])

            xa_m = [None] * n_blocks

            def emit_xa(b):
                # xa[slot, t] for lora block b, masked to own-expert slots
                g, h = divmod(b, sub)
                xa_p = xa_ps.tile([128, blk], f32, tag="xa", name=f"xa_p{b}")
                for k in range(KD):
                    nc.tensor.matmul(
                        xa_p[:],
                        lhsT=a_t[:, (b * KD + k) * 128 : (b * KD + k + 1) * 128],
                        rhs=x_t[g][:, k * GRP + h * blk : k * GRP + (h + 1) * blk],
                        start=(k == 0),
                        stop=(k == KD - 1),
                    )
                xm = cpool.tile([128, blk], bf, tag=f"xam{b}", name=f"xm{b}")
                nc.vector.tensor_mul(xm[:], xa_p[:], m_t[:, b * blk : (b + 1) * blk])
                xa_m[b] = xm

            for b in range(sub):
                emit_xa(b)

            for g in range(NG):
                for j in range(KD):
                    o_p = out_ps.tile([128, GRP], f32, tag="o", name=f"o_p{g}_{j}")
                    for k in range(KD):
                        nc.tensor.matmul(
                            o_p[:],
                            lhsT=w_t[:, k * D + j * 128 : k * D + (j + 1) * 128],
                            rhs=x_t[g][:, k * GRP : (k + 1) * GRP],
                            start=(k == 0),
                            stop=False,
                            skip_group_check=True,
                        )
                    for h in range(sub):
                        b = g * sub + h
                        nc.tensor.matmul(
                            o_p[:, h * blk : (h + 1) * blk],
                            lhsT=b_t[:, b * D + j * 128 : b * D + (j + 1) * 128],
                            rhs=xa_m[b][:],
                            start=False,
                            stop=(h == sub - 1),
                            skip_group_check=True,
                        )
                    st = stage_pool.tile([128, GRP], f32, tag="st", name=f"st{g}_{j}")
                    nc.vector.tensor_scalar_add(st[:], o_p[:], bias_t[:, j : j + 1])
                    nc.sync.dma_start(
                        out_d[j, :, g * GRP : (g + 1) * GRP], st[:]
                    )
                    if j == 3 and g < NG - 1:
                        # next group's xa, placed where its x tile has arrived
                        for h in range(sub):
                            emit_xa((g + 1) * sub + h)

    nc.compile()
    return nc


def _pick_n_blocks(labels: np.ndarray) -> int:
    for n_blocks in (8, 16, 32, 64, 128, 256):
        blk = TPC // n_blocks
        ok = True
        for c in range(N_CORES):
            sl = np.sort(labels[c * TPC : (c + 1) * TPC])
            for b in range(n_blocks):
                if len(np.unique(sl[b * blk : (b + 1) * blk])) > SLOTS:
                    ok = False
                    break
            if not ok:
                break
        if ok:
            return n_blocks
    raise ValueError("could not find a block size with <=16 experts per block")


def kernel(x, labels, W, A, B, bias):
    global _last_in_maps
    x = np.asarray(x, dtype=np.float32)
    labels_i = np.asarray(labels).astype(np.int64)
    W = np.asarray(W, dtype=np.float32)
    A = np.asarray(A, dtype=np.float32)
    B = np.asarray(B, dtype=np.float32)
    bias = np.asarray(bias, dtype=np.float32)

    n_blocks = _pick_n_blocks(labels_i)
    blk = TPC // n_blocks

    if n_blocks not in _compiled:
        _compiled[n_blocks] = _build_nc(n_blocks)
    nc = _compiled[n_blocks]

    # W[p, k, j] = W[128k+p, j]
    w_in = np.ascontiguousarray(W.reshape(KD, 128, D).transpose(1, 0, 2).astype(BF16))
    bias_in = np.ascontiguousarray(bias.reshape(KD, 128).T)  # [128, KD] f32
    B_scaled = (B * SCALING).astype(np.float32)

    in_maps = []
    perms = []
    for c in range(N_CORES):
        lc = labels_i[c * TPC : (c + 1) * TPC]
        perm = np.argsort(lc, kind="stable")
        perms.append(perm)
        ls = lc[perm]                          # sorted labels
        xs = x[c * TPC : (c + 1) * TPC][perm]  # [TPC, D] sorted tokens

        # xT[g, p, k, t] = xs[g*GRP + t, 128k + p]
        xT = np.ascontiguousarray(
            xs.astype(BF16).T.reshape(KD, 128, NG, GRP).transpose(2, 1, 0, 3)
        )

        a_in = np.zeros((128, n_blocks, KD, 128), dtype=BF16)
        b_in = np.zeros((128, n_blocks, D), dtype=BF16)
        m_in = np.zeros((128, n_blocks, blk), dtype=BF16)
        for b in range(n_blocks):
            seg = ls[b * blk : (b + 1) * blk]
            experts = np.unique(seg)
            assert len(experts) <= SLOTS
            for i, e in enumerate(experts):
                # lhsT slot: a_in[p, b, k, 8i+r] = A[e, 128k+p, r]
                a_in[:, b, :, i * R : (i + 1) * R] = A[e].reshape(KD, 128, R).transpose(
                    1, 0, 2
                )
                b_in[i * R : (i + 1) * R, b, :] = B_scaled[e]
                m_in[i * R : (i + 1) * R, b, :] = (seg == e)[None, :]

        in_maps.append(
            {
                "xT": xT,
                "W": w_in,
                "Ab": a_in,
                "Bb": b_in,
                "Mb": m_in,
                "bias": bias_in,
            }
        )

    _last_in_maps = in_maps
    res = run_bass_kernel_spmd(nc, in_maps, core_ids=list(range(N_CORES)))

    out = np.empty((T, D), dtype=np.float32)
    for c in range(N_CORES):
        o_t = res.results[c]["outT"].reshape(D, TPC)  # [d, t] sorted tokens
        out[c * TPC + perms[c]] = o_t.T
    return out


# revision 7
# speedup vs baseline: 1.3669x; 1.0017x over previous
"""MoE-LoRA linear layer (T=16384, D=1024, E=64, R=8) on 8 Trainium2 cores.

Strategy: data-parallel over tokens (2048 tokens/core). Inside each core
everything is computed transposed (d on partitions, tokens on the free dim)
so every matmul consumes operands in their natural layout with no on-device
transposes:

  out_T[:, g] = sum_k W_k^T @ xT_k[:, g]      base GEMM, N=512 token groups
  out_T[:, b] += B_blk^T @ (mask_b * (A_blk^T @ xT[:, b]))   rank-8 LoRA

Routing is resolved on the host: each core's tokens are sorted by expert
label and cut into 256-token blocks; per block the (<=16) experts present
are packed into per-block A / B / mask tensors. The device program is thus
identical for all 8 cores (one SPMD NEFF) and all data-dependence lives in
input data. The LoRA matmul accumulates directly into the base GEMM's PSUM
tile (column sub-range), so composition costs no extra DVE work.

All inputs are packed on the host into SBUF-resident layouts so each tensor
needs one large DMA (sequencer descriptor generation, ~5ns/descriptor, was
the v1 bottleneck at 161 small DMAs). Compute in bf16 (f32 PSUM): fp32
matmul on TRN2 runs at 1/4 rate and would be hopelessly PE-bound; bf16 also
halves DMA traffic.
"""

import numpy as np
import ml_dtypes

import concourse.bacc as bacc
import concourse.mybir as mybir
from concourse import tile
from concourse.bass_utils import run_bass_kernel_spmd

T, D, E, R = 16384, 1024, 64, 8
N_CORES = 8
TPC = T // N_CORES          # tokens per core
KD = D // 128               # 8 contraction chunks
GRP = 512                   # base-GEMM token group (one PSUM bank)
NG = TPC // GRP             # 4 groups
SCALING = 1.0 / R
SLOTS = 128 // R            # experts per lora block the packed layout holds

BF16 = ml_dtypes.bfloat16

_compiled = {}              # n_blocks -> Bacc program (reused across calls)
_last_in_maps = None


def _build_nc(n_blocks: int):
    blk = TPC // n_blocks   # lora block (256 default)
    sub = GRP // blk        # lora blocks per token group
    bf = mybir.dt.bfloat16
    f32 = mybir.dt.float32

    nc = bacc.Bacc(
        "TRN2", target_bir_lowering=False, debug=False, num_devices=N_CORES
    )
    # host-packed SBUF layouts, one DMA each
    xT_d = nc.dram_tensor("xT", [NG, 128, KD, GRP], bf, kind="ExternalInput")
    w_d = nc.dram_tensor("W", [128, KD, D], bf, kind="ExternalInput")
    a_d = nc.dram_tensor("Ab", [128, n_blocks, KD, 128], bf, kind="ExternalInput")
    b_d = nc.dram_tensor("Bb", [128, n_blocks, D], bf, kind="ExternalInput")
    m_d = nc.dram_tensor("Mb", [128, n_blocks, blk], bf, kind="ExternalInput")
    bias_d = nc.dram_tensor("bias", [128, KD], f32, kind="ExternalInput")
    out_d = nc.dram_tensor("outT", [KD, 128, TPC], f32, kind="ExternalOutput")

    with tile.TileContext(nc) as tc:
        with (
            tc.tile_pool(name="consts", bufs=1) as cpool,
            tc.tile_pool(name="xa_ps", bufs=2, space="PSUM") as xa_ps,
            tc.tile_pool(name="out_ps", bufs=5, space="PSUM") as out_ps,
            tc.tile_pool(name="stage", bufs=4) as stage_pool,
            tc.tile_pool(name="warm", bufs=1, space="PSUM") as warm_pool,
        ):
            KH = KD // 2  # k-chunks per half tensor (split A/x0/W DMAs so
            #               the PE can start on the first half)
            bias_t = cpool.tile([128, KD], f32, tag="bias", name="bias_t")
            a_t = [
                cpool.tile([128, n_blocks * KH * 128], bf, tag=f"a{i}", name=f"a_t{i}")
                for i in range(2)
            ]
            x_t = [
                [
                    cpool.tile([128, KH * GRP], bf, tag=f"x{g}_{i}", name=f"x_t{g}_{i}")
                    for i in range(2)
                ]
                for g in range(NG)
            ]
            w_t = [
                cpool.tile([128, KH * D], bf, tag=f"w{i}", name=f"w_t{i}")
                for i in range(2)
            ]
            m_t = cpool.tile([128, n_blocks * blk], bf, tag="m", name="m_t")
            b_t = cpool.tile([128, n_blocks * D], bf, tag="b", name="b_t")
            warm_sb = cpool.tile([128, GRP], bf, tag="warm", name="warm_sb")

            def a_sl(b, k):
                i, kk = divmod(k, KH)
                return a_t[i][:, (b * KH + kk) * 128 : (b * KH + kk + 1) * 128]

            def x_sl(g, k, c0, c1):
                i, kk = divmod(k, KH)
                return x_t[g][i][:, kk * GRP + c0 : kk * GRP + c1]

            def w_sl(k, j):
                i, kk = divmod(k, KH)
                return w_t[i][:, kk * D + j * 128 : kk * D + (j + 1) * 128]

            # issue order == arrival order (one sequencer queue): what the
            # PE needs first goes first
            nc.sync.dma_start(bias_t[:], bias_d[:, :])
            nc.sync.dma_start(a_t[0][:], a_d[:, :, 0:KH, :])
            nc.sync.dma_start(x_t[0][0][:], xT_d[0, :, 0:KH, :])
            nc.sync.dma_start(a_t[1][:], a_d[:, :, KH:KD, :])
            nc.sync.dma_start(x_t[0][1][:], xT_d[0, :, KH:KD, :])
            nc.sync.dma_start(w_t[0][:], w_d[:, 0:KH, :])
            nc.sync.dma_start(w_t[1][:], w_d[:, KH:KD, :])
            nc.sync.dma_start(m_t[:], m_d[:, :, :])
            nc.sync.dma_start(b_t[:], b_d[:, :, :])
            for g in range(1, NG):
                nc.sync.dma_start(x_t[g][0][:], xT_d[g, :, 0:KH, :])
                nc.sync.dma_start(x_t[g][1][:], xT_d[g, :, KH:KD, :])

            # PE warm-up: ~4.5us of throwaway matmuls on scratch data while
            # the input DMAs stream, so the HAM clock gate releases (1.2 ->
            # 2.4 GHz) before real work arrives and never re-throttles.
            nc.vector.memset(warm_sb[:], 0.0)
            warm_ps = warm_pool.tile([128, GRP], f32, tag="wp", name="warm_ps")
            for _ in range(22):
                nc.tensor.matmul(
                    warm_ps[:],
                    lhsT=warm_sb[:, 0:128],
                    rhs=warm_sb[:],
                    start=True,
                    stop=True,
                    skip_group_check=True,
                )

            xa_m = [None] * n_blocks

            def emit_xa(b):
                # xa[slot, t] for lora block b, masked to own-expert slots
                g, h = divmod(b, sub)
                xa_p = xa_ps.tile([128, blk], f32, tag="xa", name=f"xa_p{b}")
                for k in range(KD):
                    nc.tensor.matmul(
                        xa_p[:],
                        lhsT=a_sl(b, k),
                        rhs=x_sl(g, k, h * blk, (h + 1) * blk),
                        start=(k == 0),
                        stop=(k == KD - 1),
                    )
                xm = cpool.tile([128, blk], bf, tag=f"xam{b}", name=f"xm{b}")
                nc.vector.tensor_mul(xm[:], xa_p[:], m_t[:, b * blk : (b + 1) * blk])
                xa_m[b] = xm

            for b in range(sub):
                emit_xa(b)

            for g in range(NG):
                for j in range(KD):
                    o_p = out_ps.tile([128, GRP], f32, tag="o", name=f"o_p{g}_{j}")
                    for k in range(KD):
                        nc.tensor.matmul(
                            o_p[:],
                            lhsT=w_sl(k, j),
                            rhs=x_sl(g, k, 0, GRP),
                            start=(k == 0),
                            stop=False,
                            skip_group_check=True,
                        )
                    for h in range(sub):
                        b = g * sub + h
                        nc.tensor.matmul(
                            o_p[:, h * blk : (h + 1) * blk],
                            lhsT=b_t[:, b * D + j * 128 : b * D + (j + 1) * 128],
                            rhs=xa_m[b][:],
                            start=False,
                            stop=(h == sub - 1),
                            skip_group_check=True,
                        )
                    st = stage_pool.tile([128, GRP], f32, tag="st", name=f"st{g}_{j}")
                    nc.vector.tensor_scalar_add(st[:], o_p[:], bias_t[:, j : j + 1])
                    nc.sync.dma_start(
                        out_d[j, :, g * GRP : (g + 1) * GRP], st[:]
                    )
                    if j == 3 and g < NG - 1:
                        # next group's xa, placed where its x tile has arrived
                        for h in range(sub):
                            emit_xa((g + 1) * sub + h)

    nc.compile()
    return nc


def _pick_n_blocks(labels: np.ndarray) -> int:
    for n_blocks in (8, 16, 32, 64, 128, 256):
        blk = TPC // n_blocks
        ok = True
        for c in range(N_CORES):
            sl = np.sort(labels[c * TPC : (c + 1) * TPC])
            for b in range(n_blocks):
                if len(np.unique(sl[b * blk : (b + 1) * blk])) > SLOTS:
                    ok = False
                    break
            if not ok:
                break
        if ok:
            return n_blocks
    raise ValueError("could not find a block size with <=16 experts per block")


def kernel(x, labels, W, A, B, bias):
    global _last_in_maps
    x = np.asarray(x, dtype=np.float32)
    labels_i = np.asarray(labels).astype(np.int64)
    W = np.asarray(W, dtype=np.float32)
    A = np.asarray(A, dtype=np.float32)
    B = np.asarray(B, dtype=np.float32)
    bias = np.asarray(bias, dtype=np.float32)

    n_blocks = _pick_n_blocks(labels_i)
    blk = TPC // n_blocks

    if n_blocks not in _compiled:
        _compiled[n_blocks] = _build_nc(n_blocks)
    nc = _compiled[n_blocks]

    # W[p, k, j] = W[128k+p, j]
    w_in = np.ascontiguousarray(W.reshape(KD, 128, D).transpose(1, 0, 2).astype(BF16))
    bias_in = np.ascontiguousarray(bias.reshape(KD, 128).T)  # [128, KD] f32
    B_scaled = (B * SCALING).astype(np.float32)

    in_maps = []
    perms = []
    for c in range(N_CORES):
        lc = labels_i[c * TPC : (c + 1) * TPC]
        perm = np.argsort(lc, kind="stable")
        perms.append(perm)
        ls = lc[perm]                          # sorted labels
        xs = x[c * TPC : (c + 1) * TPC][perm]  # [TPC, D] sorted tokens

        # xT[g, p, k, t] = xs[g*GRP + t, 128k + p]
        xT = np.ascontiguousarray(
            xs.astype(BF16).T.reshape(KD, 128, NG, GRP).transpose(2, 1, 0, 3)
        )

        a_in = np.zeros((128, n_blocks, KD, 128), dtype=BF16)
        b_in = np.zeros((128, n_blocks, D), dtype=BF16)
        m_in = np.zeros((128, n_blocks, blk), dtype=BF16)
        for b in range(n_blocks):
            seg = ls[b * blk : (b + 1) * blk]
            experts = np.unique(seg)
            assert len(experts) <= SLOTS
            for i, e in enumerate(experts):
                # lhsT slot: a_in[p, b, k, 8i+r] = A[e, 128k+p, r]
                a_in[:, b, :, i * R : (i + 1) * R] = A[e].reshape(KD, 128, R).transpose(
                    1, 0, 2
                )
                b_in[i * R : (i + 1) * R, b, :] = B_scaled[e]
                m_in[i * R : (i + 1) * R, b, :] = (seg == e)[None, :]

        in_maps.append(
            {
                "xT": xT,
                "W": w_in,
                "Ab": a_in,
                "Bb": b_in,
                "Mb": m_in,
                "bias": bias_in,
            }
        )

    _last_in_maps = in_maps
    res = run_bass_kernel_spmd(nc, in_maps, core_ids=list(range(N_CORES)))

    out = np.empty((T, D), dtype=np.float32)
    for c in range(N_CORES):
        o_t = res.results[c]["outT"].reshape(D, TPC)  # [d, t] sorted tokens
        out[c * TPC + perms[c]] = o_t.T
    return out


# revision 10
# speedup vs baseline: 1.4493x; 1.0603x over previous
"""MoE-LoRA linear layer (T=16384, D=1024, E=64, R=8) on 8 Trainium2 cores.

Strategy: data-parallel over tokens (2048 tokens/core). Inside each core
everything is computed transposed (d on partitions, tokens on the free dim)
so every matmul consumes operands in their natural layout with no on-device
transposes:

  out_T[:, g] = sum_k W_k^T @ xT_k[:, g]      base GEMM, N=512 token groups
  out_T[:, b] += B_blk^T @ (mask_b * (A_blk^T @ xT[:, b]))   rank-8 LoRA

Routing is resolved on the host: each core's tokens are sorted by expert
label and cut into 256-token blocks; per block the (<=16) experts present
are packed into per-block A / B / mask tensors. The device program is thus
identical for all 8 cores (one SPMD NEFF) and all data-dependence lives in
input data. The LoRA matmul accumulates directly into the base GEMM's PSUM
tile (column sub-range), so composition costs no extra DVE work.

All inputs are packed on the host into SBUF-resident layouts so each tensor
needs one large DMA (sequencer descriptor generation, ~5ns/descriptor, was
the v1 bottleneck at 161 small DMAs). Compute in bf16 (f32 PSUM): fp32
matmul on TRN2 runs at 1/4 rate and would be hopelessly PE-bound; bf16 also
halves DMA traffic.
"""

import numpy as np
import ml_dtypes

import concourse.bacc as bacc
import concourse.mybir as mybir
from concourse import tile
from concourse.bass_utils import run_bass_kernel_spmd

T, D, E, R = 16384, 1024, 64, 8
N_CORES = 8
TPC = T // N_CORES          # tokens per core
KD = D // 128               # 8 contraction chunks
GRP = 512                   # base-GEMM token group (one PSUM bank)
NG = TPC // GRP             # 4 groups
SCALING = 1.0 / R
SLOTS = 128 // R            # experts per lora block the packed layout holds

BF16 = ml_dtypes.bfloat16

_compiled = {}              # n_blocks -> Bacc program (reused across calls)
_last_in_maps = None


def _build_nc(n_blocks: int):
    blk = TPC // n_blocks   # lora block (256 default)
    sub = GRP // blk        # lora blocks per token group
    bf = mybir.dt.bfloat16
    f32 = mybir.dt.float32

    nc = bacc.Bacc(
        "TRN2", target_bir_lowering=False, debug=False, num_devices=N_CORES
    )
    # host-packed SBUF layouts, one DMA each
    xT_d = nc.dram_tensor("xT", [NG, 128, KD, GRP], bf, kind="ExternalInput")
    w_d = nc.dram_tensor("W", [128, KD, D], bf, kind="ExternalInput")
    a_d = nc.dram_tensor("Ab", [128, n_blocks, KD, 128], bf, kind="ExternalInput")
    b_d = nc.dram_tensor("Bb", [128, n_blocks, D], bf, kind="ExternalInput")
    m_d = nc.dram_tensor("Mb", [128, n_blocks, blk], bf, kind="ExternalInput")
    bias_d = nc.dram_tensor("bias", [128, KD], f32, kind="ExternalInput")
    out_d = nc.dram_tensor("outT", [KD, 128, TPC], f32, kind="ExternalOutput")

    with tile.TileContext(nc) as tc:
        with (
            tc.tile_pool(name="consts", bufs=1) as cpool,
            tc.tile_pool(name="xa_ps", bufs=2, space="PSUM") as xa_ps,
            tc.tile_pool(name="out_ps", bufs=6, space="PSUM") as out_ps,
            tc.tile_pool(name="stage", bufs=4) as stage_pool,
        ):
            KH = KD // 2  # k-chunks per half tensor (split A/x0/W DMAs so
            #               the PE can start on the first half)
            bias_t = cpool.tile([128, KD], f32, tag="bias", name="bias_t")
            a_t = [
                cpool.tile([128, n_blocks * KH * 128], bf, tag=f"a{i}", name=f"a_t{i}")
                for i in range(2)
            ]
            x_t = [
                [
                    cpool.tile([128, KH * GRP], bf, tag=f"x{g}_{i}", name=f"x_t{g}_{i}")
                    for i in range(2)
                ]
                for g in range(NG)
            ]
            w_t = [
                cpool.tile([128, KH * D], bf, tag=f"w{i}", name=f"w_t{i}")
                for i in range(2)
            ]
            m_t = cpool.tile([128, n_blocks * blk], bf, tag="m", name="m_t")
            b_t = cpool.tile([128, n_blocks * D], bf, tag="b", name="b_t")
            warm_sb = cpool.tile([128, GRP], bf, tag="warm", name="warm_sb")

            def a_sl(b, k):
                i, kk = divmod(k, KH)
                return a_t[i][:, (b * KH + kk) * 128 : (b * KH + kk + 1) * 128]

            def x_sl(g, k, c0, c1):
                i, kk = divmod(k, KH)
                return x_t[g][i][:, kk * GRP + c0 : kk * GRP + c1]

            def w_sl(k, j):
                i, kk = divmod(k, KH)
                return w_t[i][:, kk * D + j * 128 : kk * D + (j + 1) * 128]

            # issue order == arrival order (one sequencer queue): what the
            # PE needs first goes first
            nc.sync.dma_start(bias_t[:], bias_d[:, :])
            nc.sync.dma_start(a_t[0][:], a_d[:, :, 0:KH, :])
            nc.sync.dma_start(x_t[0][0][:], xT_d[0, :, 0:KH, :])
            nc.sync.dma_start(w_t[0][:], w_d[:, 0:KH, :])
            nc.sync.dma_start(x_t[0][1][:], xT_d[0, :, KH:KD, :])
            nc.sync.dma_start(a_t[1][:], a_d[:, :, KH:KD, :])
            nc.sync.dma_start(w_t[1][:], w_d[:, KH:KD, :])
            nc.sync.dma_start(b_t[:], b_d[:, :, :])
            nc.sync.dma_start(m_t[:], m_d[:, :, :])
            for g in range(1, NG):
                nc.sync.dma_start(x_t[g][0][:], xT_d[g, :, 0:KH, :])
                nc.sync.dma_start(x_t[g][1][:], xT_d[g, :, KH:KD, :])

            # PE warm-up: throwaway matmuls on scratch while the first input
            # DMAs stream, so the HAM clock gate releases (1.2 -> 2.4 GHz)
            # before real work arrives.
            nc.vector.memset(warm_sb[:], 0.0)
            for _ in range(12):
                warm_ps = xa_ps.tile([128, GRP], f32, tag="xa", name="warm_ps")
                nc.tensor.matmul(
                    warm_ps[:],
                    lhsT=warm_sb[:, 0:128],
                    rhs=warm_sb[:],
                    start=True,
                    stop=True,
                    skip_group_check=True,
                )

            xa_m = [None] * n_blocks
            xa_p = [None] * n_blocks

            def emit_xa_half(b, half):
                # xa[slot, t] for lora block b (k-half), masked on completion
                g, h = divmod(b, sub)
                if half == 0:
                    xa_p[b] = xa_ps.tile([128, blk], f32, tag="xa", name=f"xa_p{b}")
                for kk in range(KH):
                    k = half * KH + kk
                    nc.tensor.matmul(
                        xa_p[b][:],
                        lhsT=a_sl(b, k),
                        rhs=x_sl(g, k, h * blk, (h + 1) * blk),
                        start=(k == 0),
                        stop=(k == KD - 1),
                    )
                if half == 1:
                    xm = cpool.tile([128, blk], bf, tag=f"xam{b}", name=f"xm{b}")
                    nc.vector.tensor_mul(
                        xm[:], xa_p[b][:], m_t[:, b * blk : (b + 1) * blk]
                    )
                    xa_m[b] = xm

            def emit_base(g, j, o_p, half):
                for kk in range(KH):
                    k = half * KH + kk
                    nc.tensor.matmul(
                        o_p[:],
                        lhsT=w_sl(k, j),
                        rhs=x_sl(g, k, 0, GRP),
                        start=(k == 0),
                        stop=False,
                        skip_group_check=True,
                    )

            def emit_lora_bias(g, j, o_p):
                for h in range(sub):
                    b = g * sub + h
                    nc.tensor.matmul(
                        o_p[:, h * blk : (h + 1) * blk],
                        lhsT=b_t[:, b * D + j * 128 : b * D + (j + 1) * 128],
                        rhs=xa_m[b][:],
                        start=False,
                        stop=(h == sub - 1),
                        skip_group_check=True,
                    )
                st = stage_pool.tile([128, GRP], f32, tag="st", name=f"st{g}_{j}")
                nc.vector.tensor_scalar_add(st[:], o_p[:], bias_t[:, j : j + 1])
                nc.sync.dma_start(out_d[j, :, g * GRP : (g + 1) * GRP], st[:])

            # --- group 0: k-split schedule matched to DMA arrival order ---
            # [A0,x00]   xa half-0
            for b in range(sub):
                emit_xa_half(b, 0)
            # [W0]       six j-tiles' first k-half (6 psum banks + 2 xa)
            o_p0 = {}
            for j in range(6):
                o_p0[j] = out_ps.tile([128, GRP], f32, tag="o", name=f"o_p0_{j}")
                emit_base(0, j, o_p0[j], 0)
            # [x01,A1]   xa half-1 + masks
            for b in range(sub):
                emit_xa_half(b, 1)
            # [W1]       finish the six, then j=6,7 whole
            for j in range(6):
                emit_base(0, j, o_p0[j], 1)
                emit_lora_bias(0, j, o_p0[j])
            for j in range(6, KD):
                o_p = out_ps.tile([128, GRP], f32, tag="o", name=f"o_p0_{j}")
                emit_base(0, j, o_p, 0)
                emit_base(0, j, o_p, 1)
                emit_lora_bias(0, j, o_p)
                if j == 6:
                    # group 1's xa, placed where its x tile has arrived
                    for h in range(sub):
                        emit_xa_half(sub + h, 0)
                        emit_xa_half(sub + h, 1)

            # --- groups 1..3: straight pipeline ---
            for g in range(1, NG):
                for j in range(KD):
                    o_p = out_ps.tile([128, GRP], f32, tag="o", name=f"o_p{g}_{j}")
                    emit_base(g, j, o_p, 0)
                    emit_base(g, j, o_p, 1)
                    emit_lora_bias(g, j, o_p)
                    if j == 3 and g < NG - 1:
                        # next group's xa, placed where its x tile has arrived
                        for h in range(sub):
                            emit_xa_half((g + 1) * sub + h, 0)
                            emit_xa_half((g + 1) * sub + h, 1)

    nc.compile()
    return nc


def _pick_n_blocks(labels: np.ndarray) -> int:
    for n_blocks in (8, 16, 32, 64, 128, 256):
        blk = TPC // n_blocks
        ok = True
        for c in range(N_CORES):
            sl = np.sort(labels[c * TPC : (c + 1) * TPC])
            for b in range(n_blocks):
                if len(np.unique(sl[b * blk : (b + 1) * blk])) > SLOTS:
                    ok = False
                    break
            if not ok:
                break
        if ok:
            return n_blocks
    raise ValueError("could not find a block size with <=16 experts per block")


def kernel(x, labels, W, A, B, bias):
    global _last_in_maps
    x = np.asarray(x, dtype=np.float32)
    labels_i = np.asarray(labels).astype(np.int64)
    W = np.asarray(W, dtype=np.float32)
    A = np.asarray(A, dtype=np.float32)
    B = np.asarray(B, dtype=np.float32)
    bias = np.asarray(bias, dtype=np.float32)

    n_blocks = _pick_n_blocks(labels_i)
    blk = TPC // n_blocks

    if n_blocks not in _compiled:
        _compiled[n_blocks] = _build_nc(n_blocks)
    nc = _compiled[n_blocks]

    # W[p, k, j] = W[128k+p, j]
    w_in = np.ascontiguousarray(W.reshape(KD, 128, D).transpose(1, 0, 2).astype(BF16))
    bias_in = np.ascontiguousarray(bias.reshape(KD, 128).T)  # [128, KD] f32
    B_scaled = (B * SCALING).astype(np.float32)

    in_maps = []
    perms = []
    for c in range(N_CORES):
        lc = labels_i[c * TPC : (c + 1) * TPC]
        perm = np.argsort(lc, kind="stable")
        perms.append(perm)
        ls = lc[perm]                          # sorted labels
        xs = x[c * TPC : (c + 1) * TPC][perm]  # [TPC, D] sorted tokens

        # xT[g, p, k, t] = xs[g*GRP + t, 128k + p]
        xT = np.ascontiguousarray(
            xs.astype(BF16).T.reshape(KD, 128, NG, GRP).transpose(2, 1, 0, 3)
        )

        a_in = np.zeros((128, n_blocks, KD, 128), dtype=BF16)
        b_in = np.zeros((128, n_blocks, D), dtype=BF16)
        m_in = np.zeros((128, n_blocks, blk), dtype=BF16)
        for b in range(n_blocks):
            seg = ls[b * blk : (b + 1) * blk]
            experts = np.unique(seg)
            assert len(experts) <= SLOTS
            for i, e in enumerate(experts):
                # lhsT slot: a_in[p, b, k, 8i+r] = A[e, 128k+p, r]
                a_in[:, b, :, i * R : (i + 1) * R] = A[e].reshape(KD, 128, R).transpose(
                    1, 0, 2
                )
                b_in[i * R : (i + 1) * R, b, :] = B_scaled[e]
                m_in[i * R : (i + 1) * R, b, :] = (seg == e)[None, :]

        in_maps.append(
            {
                "xT": xT,
                "W": w_in,
                "Ab": a_in,
                "Bb": b_in,
                "Mb": m_in,
                "bias": bias_in,
            }
        )

    _last_in_maps = in_maps
    res = run_bass_kernel_spmd(nc, in_maps, core_ids=list(range(N_CORES)))

    out = np.empty((T, D), dtype=np.float32)
    for c in range(N_CORES):
        o_t = res.results[c]["outT"].reshape(D, TPC)  # [d, t] sorted tokens
        out[c * TPC + perms[c]] = o_t.T
    return out
